# revision 6
# baseline (speedup 1.0000x reference)
"""Bilateral filtering kernel for Trainium2 (8 NeuronCores, SPMD).

Problem: for each image (N=4, K=3, H=W=96, P=H*W=9216):
    f_i = (x_i/100, y_i/100, rgb_i/15) in R^5
    w[i,j] = exp(-0.5 ||f_i - f_j||^2)
    out_k[i] = sum_j w[i,j] * norm_k[j]
then out /= max(out) over the whole batch.

Sharding: core c handles image c//2, output-row half c%2 (4608 rows each).

Device algorithm per core (all P x P work on-chip, never touches HBM):
  exponent arg(i,j) = f_i.f_j - 0.5|f_i|^2 - 0.5|f_j|^2  (= -0.5 d2)
  expressed as a 7-dim dot product a_i.b_j with augmented features, and
  computed in compensated bf16 (hi/lo split -> 21-dim contraction) on the
  tensor engine: 3 row-tiled concurrent matmuls (contract 21 <= 32).
  exp() on the scalar engine reading 3-bank PSUM spans (bf16).
  Stage 2 (out = v @ w) as 3 col-tiled concurrent matmuls (M=3) with
  per-column-band PSUM accumulators folded on the vector engine.
"""

import numpy as np
import ml_dtypes

bf16 = ml_dtypes.bfloat16

N_CORES = 8
N_IMG, K, H, W = 4, 3, 96, 96
P = H * W            # 9216
HALF = P // 2        # 4608
SIGMA_RGB = 15.0
SIGMA_XY = 100.0
N_JCHUNK = P // 128  # 72
EU = 3               # j-chunks (128 each) per exp unit == concurrency groups
N_UNITS = N_JCHUNK // EU  # 24
NI = 512             # i-tile width (fp32 PSUM: one matmul out <= 512 fp32)
I_TILES = [(i * NI, NI) for i in range(HALF // NI)]

_CACHE = {}


def _split_seq_waits(nc, mybir):
    """walrus on this build accepts only 1 sync wait on sequencer-only
    instructions (TPB_CTRL); split extras onto preceding drain carriers."""
    for fn in nc.m.functions:
        for bb in fn.blocks:
            insts = list(bb.instructions)
            out = []
            changed = False
            for ins in insts:
                si = ins.sync_info
                if si is not None and len(si.on_wait) > 1 and ins.is_sequencer_only:
                    waits = list(si.on_wait)
                    for w in waits[:-1]:
                        d = mybir.InstDrain(
                            name=nc.get_next_instruction_name(),
                            ins=[], outs=[], bass_is_fusable=False,
                        )
                        d.engine = ins.engine
                        d.sync_info = mybir.SyncInfo(on_wait=[w], on_update=[])
                        out.append(d)
                    ins.sync_info = mybir.SyncInfo(
                        on_wait=waits[-1:], on_update=list(si.on_update)
                    )
                    changed = True
                out.append(ins)
            if changed:
                bb.instructions = out


def _build(split_waits=True):
    import concourse.bass as bass
    import concourse.tile as tile
    from concourse import mybir

    nc = bass.Bass("TRN2", target_bir_lowering=False, debug=False,
                   num_devices=N_CORES)
    brep_d = nc.dram_tensor("Brep", [128, P], mybir.dt.bfloat16,
                            kind="ExternalInput").ap()
    arep_d = nc.dram_tensor("Arep", [128, HALF], mybir.dt.bfloat16,
                            kind="ExternalInput").ap()
    v_d = nc.dram_tensor("V", [128, N_JCHUNK, K], mybir.dt.bfloat16,
                         kind="ExternalInput").ap()
    out_d = nc.dram_tensor("out", [K, HALF], mybir.dt.float32,
                           kind="ExternalOutput").ap()

    with tile.TileContext(nc) as tc:
        with (
            tc.tile_pool(name="big", bufs=1) as big,
            tc.tile_pool(name="wpool", bufs=2) as wpool,
            tc.tile_pool(name="psum1", bufs=2, space="PSUM") as psum1,
            tc.tile_pool(name="psacc", bufs=1, space="PSUM") as psacc,
            tc.tile_pool(name="outp", bufs=4) as outp,
        ):
            brep = big.tile([128, P], mybir.dt.bfloat16)
            nc.sync.dma_start(out=brep, in_=brep_d)
            arep = big.tile([128, HALF], mybir.dt.bfloat16)
            nc.sync.dma_start(out=arep, in_=arep_d)
            vt = big.tile([128, N_JCHUNK, K], mybir.dt.bfloat16)
            nc.sync.dma_start(out=vt, in_=v_d)

            # per-column-band stage-2 accumulators: [band partitions, 512]
            accum = psacc.tile([128, NI], mybir.dt.float32)

            for (i0, wi) in I_TILES:
                for u in range(N_UNITS):
                    pe_unit = psum1.tile([128, EU, NI], mybir.dt.float32,
                                         tag="pe_unit")
                    for c in range(EU):
                        jc = u * EU + c
                        band = 32 * c
                        nc.tensor.matmul(
                            out=pe_unit[:, c, :],
                            lhsT=brep[band:band + 21, jc * 128:(jc + 1) * 128],
                            rhs=arep[band:band + 21, i0:i0 + wi],
                            start=True, stop=True,
                            tile_position=(band, 0),
                        )
                    wt = wpool.tile([128, EU, NI], mybir.dt.bfloat16,
                                    tag="wt")
                    nc.scalar.activation(
                        wt, pe_unit,
                        mybir.ActivationFunctionType.Exp,
                    )
                    for c in range(EU):
                        jc = u * EU + c
                        band = 32 * c
                        nc.tensor.matmul(
                            out=accum[band:band + K, :],
                            lhsT=vt[:, jc, :],
                            rhs=wt[:, c, :],
                            start=(u == 0), stop=(u == N_UNITS - 1),
                            tile_position=(0, band),
                            skip_group_check=True,
                        )
                # fold the 3 column bands and write out
                # (walrus: at most one PSUM operand per DVE instruction)
                t_a = outp.tile([K, NI], mybir.dt.float32, tag="fold_a")
                t_b = outp.tile([K, NI], mybir.dt.float32, tag="fold_b")
                t_c = outp.tile([K, NI], mybir.dt.float32, tag="fold_c")
                nc.vector.tensor_copy(t_a, accum[0:K, :])
                nc.vector.tensor_add(t_b, t_a, accum[32:32 + K, :])
                nc.vector.tensor_add(t_c, t_b, accum[64:64 + K, :])
                nc.sync.dma_start(out=out_d[:, i0:i0 + wi], in_=t_c)

    if split_waits:
        _split_seq_waits(nc, mybir)
    return nc


def _prep_features(raw):
    """raw [K, H, W] float32 -> (Ap, Bp) each [P, 21] bf16-exact float32."""
    ys, xs = np.meshgrid(np.arange(H, dtype=np.float64),
                         np.arange(W, dtype=np.float64), indexing="ij")
    pos = np.stack([xs.ravel(), ys.ravel()], axis=-1) / SIGMA_XY       # [P, 2]
    col = raw.reshape(K, P).T.astype(np.float64) / SIGMA_RGB           # [P, 3]
    f = np.concatenate([pos, col], axis=-1)                            # [P, 5]
    f = f - f.mean(axis=0, keepdims=True)  # d2-invariant recentring
    sq = np.sum(f * f, axis=-1)
    ones = np.ones((P, 1))
    a = np.concatenate([f, -0.5 * sq[:, None], ones], axis=-1).astype(np.float32)
    b = np.concatenate([f, ones, -0.5 * sq[:, None]], axis=-1).astype(np.float32)
    ah = a.astype(bf16)
    al = (a - ah.astype(np.float32)).astype(bf16)
    bh = b.astype(bf16)
    bl = (b - bh.astype(np.float32)).astype(bf16)
    ap = np.concatenate([ah, al, ah], axis=-1)  # [P, 21] bf16
    bp = np.concatenate([bh, bh, bl], axis=-1)  # [P, 21] bf16
    return ap, bp


def _prep_core(raw, norm, half):
    """Build per-core input map. raw/norm [K, H, W] f32."""
    ap, bp = _prep_features(raw)
    i0 = half * HALF
    brep = np.zeros((128, P), bf16)
    arep = np.zeros((128, HALF), bf16)
    for c in range(EU):
        brep[32 * c:32 * c + 21, :] = bp.T
        arep[32 * c:32 * c + 21, :] = ap[i0:i0 + HALF].T
    v = norm.reshape(K, P).astype(np.float32)          # [K, P]
    vtile = np.zeros((128, N_JCHUNK, K), bf16)
    # vtile[p, jc, k] = v[k, jc*128 + p]
    vtile[:, :, :] = v.T.reshape(N_JCHUNK, 128, K).transpose(1, 0, 2).astype(bf16)
    return {"Brep": brep, "Arep": arep, "V": vtile}


def _run(in_maps, trace=False):
    from concourse.bass_utils import run_bass_kernel_spmd

    if "nc" not in _CACHE:
        _CACHE["nc"] = _build()
    nc = _CACHE["nc"]
    return run_bass_kernel_spmd(nc, in_maps, core_ids=list(range(N_CORES)),
                                trace=trace)


def kernel(images_raw, images_normalized, _trace=False, _results=None):
    images_raw = np.asarray(images_raw, dtype=np.float32)
    images_normalized = np.asarray(images_normalized, dtype=np.float32)

    in_maps = []
    for c in range(N_CORES):
        n, h = divmod(c, 2)
        in_maps.append(_prep_core(images_raw[n], images_normalized[n], h))

    res = _run(in_maps, trace=_trace)
    if _results is not None:
        _results.append(res)

    out = np.zeros((N_IMG, K, P), np.float32)
    for c in range(N_CORES):
        n, h = divmod(c, 2)
        out[n][:, h * HALF:(h + 1) * HALF] = res.results[c]["out"]
    out /= out.max()
    return out.reshape(N_IMG, K, H, W)


# revision 9
# speedup vs baseline: 1.0211x; 1.0211x over previous
"""Bilateral filtering kernel for Trainium2 (8 NeuronCores, SPMD).

Problem: for each image (N=4, K=3, H=W=96, P=H*W=9216):
    f_i = (x_i/100, y_i/100, rgb_i/15) in R^5
    w[i,j] = exp(-0.5 ||f_i - f_j||^2)
    out_k[i] = sum_j w[i,j] * norm_k[j]
then out /= max(out) over the whole batch.

Sharding: core c handles image c//2, output-row half c%2 (4608 rows each).

Device algorithm per core (all P x P work on-chip, never touches HBM):
  exponent arg(i,j) = f_i.f_j - 0.5|f_i|^2 - 0.5|f_j|^2  (= -0.5 d2)
  expressed as a 7-dim dot product a_i.b_j with augmented features, and
  computed in compensated bf16 (hi/lo split -> 21-dim contraction) on the
  tensor engine: 3 row-tiled concurrent matmuls (contract 21 <= 32).
  exp() on the scalar engine reading 3-bank PSUM spans (bf16).
  Stage 2 (out = v @ w) as 3 col-tiled concurrent matmuls (M=3) with
  per-column-band PSUM accumulators folded on the vector engine.
"""

import numpy as np
import ml_dtypes

bf16 = ml_dtypes.bfloat16

N_CORES = 8
N_IMG, K, H, W = 4, 3, 96, 96
P = H * W            # 9216
HALF = P // 2        # 4608
SIGMA_RGB = 15.0
SIGMA_XY = 100.0
N_JCHUNK = P // 128  # 72
EU = 3               # j-chunks (128 each) per exp unit == concurrency groups
N_UNITS = N_JCHUNK // EU  # 24
NI = 512             # i-tile width (fp32 PSUM: one matmul out <= 512 fp32)
I_TILES = [(i * NI, NI) for i in range(HALF // NI)]

_CACHE = {}


def _split_seq_waits(nc, mybir):
    """walrus on this build accepts only 1 sync wait on sequencer-only
    instructions (TPB_CTRL); split extras onto preceding drain carriers."""
    for fn in nc.m.functions:
        for bb in fn.blocks:
            insts = list(bb.instructions)
            out = []
            changed = False
            for ins in insts:
                si = ins.sync_info
                if si is not None and len(si.on_wait) > 1 and ins.is_sequencer_only:
                    waits = list(si.on_wait)
                    for w in waits[:-1]:
                        d = mybir.InstDrain(
                            name=nc.get_next_instruction_name(),
                            ins=[], outs=[], bass_is_fusable=False,
                        )
                        d.engine = ins.engine
                        d.sync_info = mybir.SyncInfo(on_wait=[w], on_update=[])
                        out.append(d)
                    ins.sync_info = mybir.SyncInfo(
                        on_wait=waits[-1:], on_update=list(si.on_update)
                    )
                    changed = True
                out.append(ins)
            if changed:
                bb.instructions = out


def _build(split_waits=True):
    import concourse.bass as bass
    import concourse.tile as tile
    from concourse import mybir

    nc = bass.Bass("TRN2", target_bir_lowering=False, debug=False,
                   num_devices=N_CORES)
    brep_d = nc.dram_tensor("Brep", [128, P], mybir.dt.bfloat16,
                            kind="ExternalInput").ap()
    arep_d = nc.dram_tensor("Arep", [128, HALF], mybir.dt.bfloat16,
                            kind="ExternalInput").ap()
    v_d = nc.dram_tensor("V", [128, N_JCHUNK, K], mybir.dt.bfloat16,
                         kind="ExternalInput").ap()
    out_d = nc.dram_tensor("out", [K, HALF], mybir.dt.float32,
                           kind="ExternalOutput").ap()

    with tile.TileContext(nc) as tc:
        with (
            tc.tile_pool(name="big", bufs=1) as big,
            tc.tile_pool(name="wpool", bufs=3) as wpool,
            tc.tile_pool(name="psum1", bufs=2, space="PSUM") as psum1,
            tc.tile_pool(name="psacc", bufs=1, space="PSUM") as psacc,
            tc.tile_pool(name="outp", bufs=4) as outp,
        ):
            # split input DMAs so the first units' operands land early
            arep = big.tile([128, HALF], mybir.dt.bfloat16)
            nc.sync.dma_start(out=arep[:, 0:NI], in_=arep_d[:, 0:NI])
            brep = big.tile([128, P], mybir.dt.bfloat16)
            for lo, hi in ((0, 2304), (2304, 4608), (4608, 6912), (6912, P)):
                nc.sync.dma_start(out=brep[:, lo:hi], in_=brep_d[:, lo:hi])
            nc.sync.dma_start(out=arep[:, NI:], in_=arep_d[:, NI:])
            vt = big.tile([128, N_JCHUNK, K], mybir.dt.bfloat16)
            nc.sync.dma_start(out=vt, in_=v_d)

            # per-column-band stage-2 accumulators: [band partitions, 512]
            accum = psacc.tile([128, NI], mybir.dt.float32)

            def emit_s2(u, wt):
                for c in range(EU):
                    jc = u * EU + c
                    band = 32 * c
                    nc.tensor.matmul(
                        out=accum[band:band + K, :],
                        lhsT=vt[:, jc, :],
                        rhs=wt[:, c, :],
                        start=(u == 0), stop=(u == N_UNITS - 1),
                        tile_position=(0, band),
                        skip_group_check=True,
                    )

            def emit_fold(i0):
                # fold the 3 column bands and write out
                # (walrus: at most one PSUM operand per DVE instruction)
                t_a = outp.tile([K, NI], mybir.dt.float32, tag="fold_a")
                t_b = outp.tile([K, NI], mybir.dt.float32, tag="fold_b")
                t_c = outp.tile([K, NI], mybir.dt.float32, tag="fold_c")
                nc.vector.tensor_copy(t_a, accum[0:K, :])
                nc.vector.tensor_add(t_b, t_a, accum[32:32 + K, :])
                nc.vector.tensor_add(t_c, t_b, accum[64:64 + K, :])
                nc.sync.dma_start(out=out_d[:, i0:i0 + NI], in_=t_c)

            # stage-2 of unit g is emitted during step g+1 so the PE never
            # blocks the stage-1 matmuls that feed the next exp
            pending = None  # (u, wt, i0)
            for it, (i0, wi) in enumerate(I_TILES):
                for u in range(N_UNITS):
                    pe_unit = psum1.tile([128, EU, NI], mybir.dt.float32,
                                         tag="pe_unit")
                    for c in range(EU):
                        jc = u * EU + c
                        band = 32 * c
                        nc.tensor.matmul(
                            out=pe_unit[:, c, :],
                            lhsT=brep[band:band + 21, jc * 128:(jc + 1) * 128],
                            rhs=arep[band:band + 21, i0:i0 + wi],
                            start=True, stop=True,
                            tile_position=(band, 0),
                        )
                    wt = wpool.tile([128, EU, NI], mybir.dt.bfloat16,
                                    tag="wt")
                    nc.scalar.activation(
                        wt, pe_unit,
                        mybir.ActivationFunctionType.Exp,
                    )
                    if pending is not None:
                        pu, pwt, pi0 = pending
                        emit_s2(pu, pwt)
                        if pu == N_UNITS - 1:
                            emit_fold(pi0)
                    pending = (u, wt, i0)
            pu, pwt, pi0 = pending
            emit_s2(pu, pwt)
            emit_fold(pi0)

    if split_waits:
        _split_seq_waits(nc, mybir)
    return nc


def _prep_features(raw):
    """raw [K, H, W] float32 -> (Ap, Bp) each [P, 21] bf16-exact float32."""
    ys, xs = np.meshgrid(np.arange(H, dtype=np.float64),
                         np.arange(W, dtype=np.float64), indexing="ij")
    pos = np.stack([xs.ravel(), ys.ravel()], axis=-1) / SIGMA_XY       # [P, 2]
    col = raw.reshape(K, P).T.astype(np.float64) / SIGMA_RGB           # [P, 3]
    f = np.concatenate([pos, col], axis=-1)                            # [P, 5]
    f = f - f.mean(axis=0, keepdims=True)  # d2-invariant recentring
    sq = np.sum(f * f, axis=-1)
    ones = np.ones((P, 1))
    a = np.concatenate([f, -0.5 * sq[:, None], ones], axis=-1).astype(np.float32)
    b = np.concatenate([f, ones, -0.5 * sq[:, None]], axis=-1).astype(np.float32)
    ah = a.astype(bf16)
    al = (a - ah.astype(np.float32)).astype(bf16)
    bh = b.astype(bf16)
    bl = (b - bh.astype(np.float32)).astype(bf16)
    ap = np.concatenate([ah, al, ah], axis=-1)  # [P, 21] bf16
    bp = np.concatenate([bh, bh, bl], axis=-1)  # [P, 21] bf16
    return ap, bp


def _prep_core(raw, norm, half):
    """Build per-core input map. raw/norm [K, H, W] f32."""
    ap, bp = _prep_features(raw)
    i0 = half * HALF
    brep = np.zeros((128, P), bf16)
    arep = np.zeros((128, HALF), bf16)
    for c in range(EU):
        brep[32 * c:32 * c + 21, :] = bp.T
        arep[32 * c:32 * c + 21, :] = ap[i0:i0 + HALF].T
    v = norm.reshape(K, P).astype(np.float32)          # [K, P]
    vtile = np.zeros((128, N_JCHUNK, K), bf16)
    # vtile[p, jc, k] = v[k, jc*128 + p]
    vtile[:, :, :] = v.T.reshape(N_JCHUNK, 128, K).transpose(1, 0, 2).astype(bf16)
    return {"Brep": brep, "Arep": arep, "V": vtile}


def _run(in_maps, trace=False):
    from concourse.bass_utils import run_bass_kernel_spmd

    if "nc" not in _CACHE:
        _CACHE["nc"] = _build()
    nc = _CACHE["nc"]
    return run_bass_kernel_spmd(nc, in_maps, core_ids=list(range(N_CORES)),
                                trace=trace)


def kernel(images_raw, images_normalized, _trace=False, _results=None):
    images_raw = np.asarray(images_raw, dtype=np.float32)
    images_normalized = np.asarray(images_normalized, dtype=np.float32)

    in_maps = []
    for c in range(N_CORES):
        n, h = divmod(c, 2)
        in_maps.append(_prep_core(images_raw[n], images_normalized[n], h))

    res = _run(in_maps, trace=_trace)
    if _results is not None:
        _results.append(res)

    out = np.zeros((N_IMG, K, P), np.float32)
    for c in range(N_CORES):
        n, h = divmod(c, 2)
        out[n][:, h * HALF:(h + 1) * HALF] = res.results[c]["out"]
    out /= out.max()
    return out.reshape(N_IMG, K, H, W)


# revision 10
# speedup vs baseline: 1.0376x; 1.0161x over previous
"""Bilateral filtering kernel for Trainium2 (8 NeuronCores, SPMD).

Problem: for each image (N=4, K=3, H=W=96, P=H*W=9216):
    f_i = (x_i/100, y_i/100, rgb_i/15) in R^5
    w[i,j] = exp(-0.5 ||f_i - f_j||^2)
    out_k[i] = sum_j w[i,j] * norm_k[j]
then out /= max(out) over the whole batch.

Sharding: core c handles image c//2, output-row half c%2 (4608 rows each).

Device algorithm per core (all P x P work on-chip, never touches HBM):
  exponent arg(i,j) = f_i.f_j - 0.5|f_i|^2 - 0.5|f_j|^2  (= -0.5 d2)
  expressed as a 7-dim dot product a_i.b_j with augmented features, and
  computed in compensated bf16 (hi/lo split -> 21-dim contraction) on the
  tensor engine: 3 row-tiled concurrent matmuls (contract 21 <= 32).
  exp() on the scalar engine reading 3-bank PSUM spans (bf16).
  Stage 2 (out = v @ w) as 3 col-tiled concurrent matmuls (M=3) with
  per-column-band PSUM accumulators folded on the vector engine.
"""

import numpy as np
import ml_dtypes

bf16 = ml_dtypes.bfloat16

N_CORES = 8
N_IMG, K, H, W = 4, 3, 96, 96
P = H * W            # 9216
HALF = P // 2        # 4608
SIGMA_RGB = 15.0
SIGMA_XY = 100.0
N_JCHUNK = P // 128  # 72
EU = 3               # j-chunks (128 each) per exp unit == concurrency groups
N_UNITS = N_JCHUNK // EU  # 24
NI = 512             # i-tile width (fp32 PSUM: one matmul out <= 512 fp32)
I_TILES = [(i * NI, NI) for i in range(HALF // NI)]

_CACHE = {}


def _split_seq_waits(nc, mybir):
    """walrus on this build accepts only 1 sync wait on sequencer-only
    instructions (TPB_CTRL); split extras onto preceding drain carriers."""
    for fn in nc.m.functions:
        for bb in fn.blocks:
            insts = list(bb.instructions)
            out = []
            changed = False
            for ins in insts:
                si = ins.sync_info
                if si is not None and len(si.on_wait) > 1 and ins.is_sequencer_only:
                    waits = list(si.on_wait)
                    for w in waits[:-1]:
                        d = mybir.InstDrain(
                            name=nc.get_next_instruction_name(),
                            ins=[], outs=[], bass_is_fusable=False,
                        )
                        d.engine = ins.engine
                        d.sync_info = mybir.SyncInfo(on_wait=[w], on_update=[])
                        out.append(d)
                    ins.sync_info = mybir.SyncInfo(
                        on_wait=waits[-1:], on_update=list(si.on_update)
                    )
                    changed = True
                out.append(ins)
            if changed:
                bb.instructions = out


def _build(split_waits=True):
    import concourse.bass as bass
    import concourse.tile as tile
    from concourse import mybir

    nc = bass.Bass("TRN2", target_bir_lowering=False, debug=False,
                   num_devices=N_CORES)
    brep_d = nc.dram_tensor("Brep", [128, P], mybir.dt.bfloat16,
                            kind="ExternalInput").ap()
    arep_d = nc.dram_tensor("Arep", [128, HALF], mybir.dt.bfloat16,
                            kind="ExternalInput").ap()
    v_d = nc.dram_tensor("V", [128, N_JCHUNK, K], mybir.dt.bfloat16,
                         kind="ExternalInput").ap()
    out_d = nc.dram_tensor("out", [K, HALF], mybir.dt.float32,
                           kind="ExternalOutput").ap()

    with tile.TileContext(nc) as tc:
        with (
            tc.tile_pool(name="big", bufs=1) as big,
            tc.tile_pool(name="wpool", bufs=3) as wpool,
            tc.tile_pool(name="psum1", bufs=2, space="PSUM") as psum1,
            tc.tile_pool(name="psacc", bufs=1, space="PSUM") as psacc,
            tc.tile_pool(name="outp", bufs=4) as outp,
        ):
            # split input DMAs so the first units' operands land early
            arep = big.tile([128, HALF], mybir.dt.bfloat16)
            nc.sync.dma_start(out=arep[:, 0:NI], in_=arep_d[:, 0:NI])
            brep = big.tile([128, P], mybir.dt.bfloat16)
            nc.sync.dma_start(out=brep[:, 0:1152], in_=brep_d[:, 0:1152])
            vt = big.tile([128, N_JCHUNK, K], mybir.dt.bfloat16)
            nc.sync.dma_start(out=vt, in_=v_d)
            for lo in range(1152, P, 1152):
                nc.sync.dma_start(out=brep[:, lo:lo + 1152],
                                  in_=brep_d[:, lo:lo + 1152])
            nc.sync.dma_start(out=arep[:, NI:], in_=arep_d[:, NI:])

            # per-column-band stage-2 accumulators: [band partitions, 512]
            accum = psacc.tile([128, NI], mybir.dt.float32)

            def emit_s2(u, wt):
                for c in range(EU):
                    jc = u * EU + c
                    band = 32 * c
                    nc.tensor.matmul(
                        out=accum[band:band + K, :],
                        lhsT=vt[:, jc, :],
                        rhs=wt[:, c, :],
                        start=(u == 0), stop=(u == N_UNITS - 1),
                        tile_position=(0, band),
                        skip_group_check=True,
                    )

            def emit_fold(i0):
                # fold the 3 column bands and write out
                # (walrus: at most one PSUM operand per DVE instruction)
                t_a = outp.tile([K, NI], mybir.dt.float32, tag="fold_a")
                t_b = outp.tile([K, NI], mybir.dt.float32, tag="fold_b")
                t_c = outp.tile([K, NI], mybir.dt.float32, tag="fold_c")
                nc.vector.tensor_copy(t_a, accum[0:K, :])
                nc.vector.tensor_add(t_b, t_a, accum[32:32 + K, :])
                nc.vector.tensor_add(t_c, t_b, accum[64:64 + K, :])
                nc.sync.dma_start(out=out_d[:, i0:i0 + NI], in_=t_c)

            # stage-2 of unit g is emitted during step g+1 so the PE never
            # blocks the stage-1 matmuls that feed the next exp
            pending = None  # (u, wt, i0)
            for it, (i0, wi) in enumerate(I_TILES):
                for u in range(N_UNITS):
                    pe_unit = psum1.tile([128, EU, NI], mybir.dt.float32,
                                         tag="pe_unit")
                    for c in range(EU):
                        jc = u * EU + c
                        band = 32 * c
                        nc.tensor.matmul(
                            out=pe_unit[:, c, :],
                            lhsT=brep[band:band + 21, jc * 128:(jc + 1) * 128],
                            rhs=arep[band:band + 21, i0:i0 + wi],
                            start=True, stop=True,
                            tile_position=(band, 0),
                        )
                    wt = wpool.tile([128, EU, NI], mybir.dt.bfloat16,
                                    tag="wt")
                    nc.scalar.activation(
                        wt, pe_unit,
                        mybir.ActivationFunctionType.Exp,
                    )
                    if pending is not None:
                        pu, pwt, pi0 = pending
                        emit_s2(pu, pwt)
                        if pu == N_UNITS - 1:
                            emit_fold(pi0)
                    pending = (u, wt, i0)
            pu, pwt, pi0 = pending
            emit_s2(pu, pwt)
            emit_fold(pi0)

    if split_waits:
        _split_seq_waits(nc, mybir)
    return nc


def _prep_features(raw):
    """raw [K, H, W] float32 -> (Ap, Bp) each [P, 21] bf16-exact float32."""
    ys, xs = np.meshgrid(np.arange(H, dtype=np.float64),
                         np.arange(W, dtype=np.float64), indexing="ij")
    pos = np.stack([xs.ravel(), ys.ravel()], axis=-1) / SIGMA_XY       # [P, 2]
    col = raw.reshape(K, P).T.astype(np.float64) / SIGMA_RGB           # [P, 3]
    f = np.concatenate([pos, col], axis=-1)                            # [P, 5]
    f = f - f.mean(axis=0, keepdims=True)  # d2-invariant recentring
    sq = np.sum(f * f, axis=-1)
    ones = np.ones((P, 1))
    a = np.concatenate([f, -0.5 * sq[:, None], ones], axis=-1).astype(np.float32)
    b = np.concatenate([f, ones, -0.5 * sq[:, None]], axis=-1).astype(np.float32)
    ah = a.astype(bf16)
    al = (a - ah.astype(np.float32)).astype(bf16)
    bh = b.astype(bf16)
    bl = (b - bh.astype(np.float32)).astype(bf16)
    ap = np.concatenate([ah, al, ah], axis=-1)  # [P, 21] bf16
    bp = np.concatenate([bh, bh, bl], axis=-1)  # [P, 21] bf16
    return ap, bp


def _prep_core(raw, norm, half):
    """Build per-core input map. raw/norm [K, H, W] f32."""
    ap, bp = _prep_features(raw)
    i0 = half * HALF
    brep = np.zeros((128, P), bf16)
    arep = np.zeros((128, HALF), bf16)
    for c in range(EU):
        brep[32 * c:32 * c + 21, :] = bp.T
        arep[32 * c:32 * c + 21, :] = ap[i0:i0 + HALF].T
    v = norm.reshape(K, P).astype(np.float32)          # [K, P]
    vtile = np.zeros((128, N_JCHUNK, K), bf16)
    # vtile[p, jc, k] = v[k, jc*128 + p]
    vtile[:, :, :] = v.T.reshape(N_JCHUNK, 128, K).transpose(1, 0, 2).astype(bf16)
    return {"Brep": brep, "Arep": arep, "V": vtile}


def _run(in_maps, trace=False):
    from concourse.bass_utils import run_bass_kernel_spmd

    if "nc" not in _CACHE:
        _CACHE["nc"] = _build()
    nc = _CACHE["nc"]
    return run_bass_kernel_spmd(nc, in_maps, core_ids=list(range(N_CORES)),
                                trace=trace)


def kernel(images_raw, images_normalized, _trace=False, _results=None):
    images_raw = np.asarray(images_raw, dtype=np.float32)
    images_normalized = np.asarray(images_normalized, dtype=np.float32)

    in_maps = []
    for c in range(N_CORES):
        n, h = divmod(c, 2)
        in_maps.append(_prep_core(images_raw[n], images_normalized[n], h))

    res = _run(in_maps, trace=_trace)
    if _results is not None:
        _results.append(res)

    out = np.zeros((N_IMG, K, P), np.float32)
    for c in range(N_CORES):
        n, h = divmod(c, 2)
        out[n][:, h * HALF:(h + 1) * HALF] = res.results[c]["out"]
    out /= out.max()
    return out.reshape(N_IMG, K, H, W)


# revision 17
# speedup vs baseline: 1.0410x; 1.0033x over previous
"""Bilateral filtering kernel for Trainium2 (8 NeuronCores, SPMD).

Problem: for each image (N=4, K=3, H=W=96, P=H*W=9216):
    f_i = (x_i/100, y_i/100, rgb_i/15) in R^5
    w[i,j] = exp(-0.5 ||f_i - f_j||^2)
    out_k[i] = sum_j w[i,j] * norm_k[j]
then out /= max(out) over the whole batch.

Sharding: core c handles image c//2, output-row half c%2 (4608 rows each).

Device algorithm per core (all P x P work on-chip, never touches HBM):
  exponent arg(i,j) = f_i.f_j - 0.5|f_i|^2 - 0.5|f_j|^2  (= -0.5 d2)
  expressed as a 7-dim dot product a_i.b_j with augmented features, and
  computed in compensated bf16 (hi/lo split -> 21-dim contraction) on the
  tensor engine: 3 row-tiled matmuls (contract 21 <= 32) per unit.
  exp() on the scalar engine reads 3-bank fp32 PSUM spans (1536 elems)
  and writes bf16 W tiles to SBUF; the scalar engine is the bottleneck
  (42.5M exps/core at 128 lanes @ 1.2 GHz ~= 332us of the ~370us total).
  Stage 2 (out = v @ w) as 3 col-tiled matmuls (M=3) accumulating into
  per-column-band regions of one PSUM bank, folded on the vector engine.
"""

import numpy as np
import ml_dtypes

bf16 = ml_dtypes.bfloat16

N_CORES = 8
N_IMG, K, H, W = 4, 3, 96, 96
P = H * W            # 9216
HALF = P // 2        # 4608
SIGMA_RGB = 15.0
SIGMA_XY = 100.0
N_JCHUNK = P // 128  # 72
# exp-unit sizes per i-tile (j-chunks per scalar-engine exp instruction).
# Asymmetric [4,3] pairs use 4+3+1(accum) = 8 PSUM banks and give bigger
# activation spans (2048/1536) than uniform 3s, amortizing the ~312-cycle
# per-instruction ACT overhead.
UNIT_SIZES = [4, 3] * 10 + [2]   # sums to 72
NI = 512             # i-tile width (fp32 PSUM: one matmul out <= 512 fp32)
I_TILES = [(i * NI, NI) for i in range(HALF // NI)]

_CACHE = {}


def _split_seq_waits(nc, mybir):
    """walrus on this build accepts only 1 sync wait on sequencer-only
    instructions (TPB_CTRL); split extras onto preceding drain carriers."""
    for fn in nc.m.functions:
        for bb in fn.blocks:
            insts = list(bb.instructions)
            out = []
            changed = False
            for ins in insts:
                si = ins.sync_info
                if si is not None and len(si.on_wait) > 1 and ins.is_sequencer_only:
                    waits = list(si.on_wait)
                    for w in waits[:-1]:
                        d = mybir.InstDrain(
                            name=nc.get_next_instruction_name(),
                            ins=[], outs=[], bass_is_fusable=False,
                        )
                        d.engine = ins.engine
                        d.sync_info = mybir.SyncInfo(on_wait=[w], on_update=[])
                        out.append(d)
                    ins.sync_info = mybir.SyncInfo(
                        on_wait=waits[-1:], on_update=list(si.on_update)
                    )
                    changed = True
                out.append(ins)
            if changed:
                bb.instructions = out


def _build(split_waits=True):
    import concourse.bass as bass
    import concourse.tile as tile
    from concourse import mybir

    nc = bass.Bass("TRN2", target_bir_lowering=False, debug=False,
                   num_devices=N_CORES)
    brep_d = nc.dram_tensor("Brep", [128, P], mybir.dt.bfloat16,
                            kind="ExternalInput").ap()
    arep_d = nc.dram_tensor("Arep", [128, HALF], mybir.dt.bfloat16,
                            kind="ExternalInput").ap()
    v_d = nc.dram_tensor("V", [128, N_JCHUNK, K], mybir.dt.bfloat16,
                         kind="ExternalInput").ap()
    out_d = nc.dram_tensor("out", [K, HALF], mybir.dt.float32,
                           kind="ExternalOutput").ap()

    with tile.TileContext(nc) as tc:
        with (
            tc.tile_pool(name="big", bufs=1) as big,
            tc.tile_pool(name="wpool", bufs=3) as wpool,
            tc.tile_pool(name="psumA", bufs=1, space="PSUM") as psumA,
            tc.tile_pool(name="psumB", bufs=1, space="PSUM") as psumB,
            tc.tile_pool(name="psacc", bufs=1, space="PSUM") as psacc,
            tc.tile_pool(name="outp", bufs=4) as outp,
        ):
            # split input DMAs so the first units' operands land early
            arep = big.tile([128, HALF], mybir.dt.bfloat16)
            nc.sync.dma_start(out=arep[:, 0:NI], in_=arep_d[:, 0:NI])
            brep = big.tile([128, P], mybir.dt.bfloat16)
            nc.sync.dma_start(out=brep[:, 0:1152], in_=brep_d[:, 0:1152])
            vt = big.tile([128, N_JCHUNK, K], mybir.dt.bfloat16)
            nc.sync.dma_start(out=vt, in_=v_d)
            for lo in range(1152, P, 1152):
                nc.sync.dma_start(out=brep[:, lo:lo + 1152],
                                  in_=brep_d[:, lo:lo + 1152])
            nc.sync.dma_start(out=arep[:, NI:], in_=arep_d[:, NI:])

            # per-column-band stage-2 accumulators: [band partitions, 512]
            accum = psacc.tile([128, NI], mybir.dt.float32)

            def emit_s2(jc0, size, wt):
                for idx in range(size):
                    jc = jc0 + idx
                    band = 32 * (jc % 3)   # accumulator column band
                    nc.tensor.matmul(
                        out=accum[band:band + K, :],
                        lhsT=vt[:, jc, :],
                        rhs=wt[:, idx, :],
                        start=(jc < 3), stop=(jc >= N_JCHUNK - 3),
                        tile_position=(0, band),
                        skip_group_check=True,
                    )

            def emit_fold(i0):
                # fold the 3 column bands and write out
                # (walrus: at most one PSUM operand per DVE instruction)
                t_a = outp.tile([K, NI], mybir.dt.float32, tag="fold_a")
                t_b = outp.tile([K, NI], mybir.dt.float32, tag="fold_b")
                t_c = outp.tile([K, NI], mybir.dt.float32, tag="fold_c")
                nc.vector.tensor_copy(t_a, accum[0:K, :])
                nc.vector.tensor_add(t_b, t_a, accum[32:32 + K, :])
                nc.vector.tensor_add(t_c, t_b, accum[64:64 + K, :])
                nc.sync.dma_start(out=out_d[:, i0:i0 + NI], in_=t_c)

            # stage-2 of unit g is emitted during step g+1 so the PE never
            # blocks the stage-1 matmuls that feed the next exp
            pending = None  # (jc0, size, wt, i0, last_in_itile)
            for it, (i0, wi) in enumerate(I_TILES):
                jc0 = 0
                for u, size in enumerate(UNIT_SIZES):
                    if size == 4:
                        pe_unit = psumA.tile([128, 4, NI], mybir.dt.float32,
                                             tag="unitA")
                    else:
                        pe_unit = psumB.tile([128, 3, NI], mybir.dt.float32,
                                             tag="unitB")
                    for c in range(size):
                        jc = jc0 + c
                        band = 32 * c
                        nc.tensor.matmul(
                            out=pe_unit[:, c, :],
                            lhsT=brep[band:band + 21, jc * 128:(jc + 1) * 128],
                            rhs=arep[band:band + 21, i0:i0 + wi],
                            start=True, stop=True,
                            tile_position=(band, 0),
                        )
                    wt = wpool.tile([128, 4, NI], mybir.dt.bfloat16,
                                    tag="wt")
                    nc.scalar.activation(
                        wt[:, :size, :], pe_unit[:, :size, :],
                        mybir.ActivationFunctionType.Exp,
                    )
                    if pending is not None:
                        pjc0, psize, pwt, pi0, plast = pending
                        emit_s2(pjc0, psize, pwt)
                        if plast:
                            emit_fold(pi0)
                    pending = (jc0, size, wt, i0,
                               u == len(UNIT_SIZES) - 1)
                    jc0 += size
            pjc0, psize, pwt, pi0, plast = pending
            emit_s2(pjc0, psize, pwt)
            emit_fold(pi0)

    if split_waits:
        _split_seq_waits(nc, mybir)
    return nc


def _prep_features(raw):
    """raw [K, H, W] float32 -> (Ap, Bp) each [P, 21] bf16-exact float32."""
    ys, xs = np.meshgrid(np.arange(H, dtype=np.float64),
                         np.arange(W, dtype=np.float64), indexing="ij")
    pos = np.stack([xs.ravel(), ys.ravel()], axis=-1) / SIGMA_XY       # [P, 2]
    col = raw.reshape(K, P).T.astype(np.float64) / SIGMA_RGB           # [P, 3]
    f = np.concatenate([pos, col], axis=-1)                            # [P, 5]
    f = f - f.mean(axis=0, keepdims=True)  # d2-invariant recentring
    sq = np.sum(f * f, axis=-1)
    ones = np.ones((P, 1))
    a = np.concatenate([f, -0.5 * sq[:, None], ones], axis=-1).astype(np.float32)
    b = np.concatenate([f, ones, -0.5 * sq[:, None]], axis=-1).astype(np.float32)
    ah = a.astype(bf16)
    al = (a - ah.astype(np.float32)).astype(bf16)
    bh = b.astype(bf16)
    bl = (b - bh.astype(np.float32)).astype(bf16)
    ap = np.concatenate([ah, al, ah], axis=-1)  # [P, 21] bf16
    bp = np.concatenate([bh, bh, bl], axis=-1)  # [P, 21] bf16
    return ap, bp


def _prep_core(raw, norm, half):
    """Build per-core input map. raw/norm [K, H, W] f32."""
    ap, bp = _prep_features(raw)
    i0 = half * HALF
    brep = np.zeros((128, P), bf16)
    arep = np.zeros((128, HALF), bf16)
    for c in range(4):  # replicate at all four 32-partition row bands
        brep[32 * c:32 * c + 21, :] = bp.T
        arep[32 * c:32 * c + 21, :] = ap[i0:i0 + HALF].T
    v = norm.reshape(K, P).astype(np.float32)          # [K, P]
    vtile = np.zeros((128, N_JCHUNK, K), bf16)
    # vtile[p, jc, k] = v[k, jc*128 + p]
    vtile[:, :, :] = v.T.reshape(N_JCHUNK, 128, K).transpose(1, 0, 2).astype(bf16)
    return {"Brep": brep, "Arep": arep, "V": vtile}


def _run(in_maps, trace=False):
    from concourse.bass_utils import run_bass_kernel_spmd

    if "nc" not in _CACHE:
        _CACHE["nc"] = _build()
    nc = _CACHE["nc"]
    return run_bass_kernel_spmd(nc, in_maps, core_ids=list(range(N_CORES)),
                                trace=trace)


def kernel(images_raw, images_normalized, _trace=False, _results=None):
    images_raw = np.asarray(images_raw, dtype=np.float32)
    images_normalized = np.asarray(images_normalized, dtype=np.float32)

    in_maps = []
    for c in range(N_CORES):
        n, h = divmod(c, 2)
        in_maps.append(_prep_core(images_raw[n], images_normalized[n], h))

    res = _run(in_maps, trace=_trace)
    if _results is not None:
        _results.append(res)

    out = np.zeros((N_IMG, K, P), np.float32)
    for c in range(N_CORES):
        n, h = divmod(c, 2)
        out[n][:, h * HALF:(h + 1) * HALF] = res.results[c]["out"]
    out /= out.max()
    return out.reshape(N_IMG, K, H, W)


# revision 18
# speedup vs baseline: 1.0447x; 1.0036x over previous
"""Bilateral filtering kernel for Trainium2 (8 NeuronCores, SPMD).

Problem: for each image (N=4, K=3, H=W=96, P=H*W=9216):
    f_i = (x_i/100, y_i/100, rgb_i/15) in R^5
    w[i,j] = exp(-0.5 ||f_i - f_j||^2)
    out_k[i] = sum_j w[i,j] * norm_k[j]
then out /= max(out) over the whole batch.

Sharding: core c handles image c//2, output-row half c%2 (4608 rows each).

Device algorithm per core (all P x P work on-chip, never touches HBM):
  exponent arg(i,j) = f_i.f_j - 0.5|f_i|^2 - 0.5|f_j|^2  (= -0.5 d2)
  expressed as a 7-dim dot product a_i.b_j with augmented features, and
  computed in compensated bf16 (hi/lo split -> 21-dim contraction) on the
  tensor engine: 3 row-tiled matmuls (contract 21 <= 32) per unit.
  exp() on the scalar engine reads 3-bank fp32 PSUM spans (1536 elems)
  and writes bf16 W tiles to SBUF; the scalar engine is the bottleneck
  (42.5M exps/core at 128 lanes @ 1.2 GHz ~= 332us of the ~370us total).
  Stage 2 (out = v @ w) as 3 col-tiled matmuls (M=3) accumulating into
  per-column-band regions of one PSUM bank, folded on the vector engine.
"""

import numpy as np
import ml_dtypes

bf16 = ml_dtypes.bfloat16

N_CORES = 8
N_IMG, K, H, W = 4, 3, 96, 96
P = H * W            # 9216
HALF = P // 2        # 4608
SIGMA_RGB = 15.0
SIGMA_XY = 100.0
N_JCHUNK = P // 128  # 72
# exp-unit sizes per i-tile (j-chunks per scalar-engine exp instruction).
# Asymmetric [4,3] pairs use 4+3+1(accum) = 8 PSUM banks and give bigger
# activation spans (2048/1536) than uniform 3s, amortizing the ~312-cycle
# per-instruction ACT overhead.
UNIT_SIZES = [4, 3] * 10 + [2]   # sums to 72
NI = 512             # i-tile width (fp32 PSUM: one matmul out <= 512 fp32)
I_TILES = [(i * NI, NI) for i in range(HALF // NI)]

_CACHE = {}


def _split_seq_waits(nc, mybir):
    """walrus on this build accepts only 1 sync wait on sequencer-only
    instructions (TPB_CTRL); split extras onto preceding drain carriers."""
    for fn in nc.m.functions:
        for bb in fn.blocks:
            insts = list(bb.instructions)
            out = []
            changed = False
            for ins in insts:
                si = ins.sync_info
                if si is not None and len(si.on_wait) > 1 and ins.is_sequencer_only:
                    waits = list(si.on_wait)
                    for w in waits[:-1]:
                        d = mybir.InstDrain(
                            name=nc.get_next_instruction_name(),
                            ins=[], outs=[], bass_is_fusable=False,
                        )
                        d.engine = ins.engine
                        d.sync_info = mybir.SyncInfo(on_wait=[w], on_update=[])
                        out.append(d)
                    ins.sync_info = mybir.SyncInfo(
                        on_wait=waits[-1:], on_update=list(si.on_update)
                    )
                    changed = True
                out.append(ins)
            if changed:
                bb.instructions = out


def _build(split_waits=True):
    import concourse.bass as bass
    import concourse.tile as tile
    from concourse import mybir

    nc = bass.Bass("TRN2", target_bir_lowering=False, debug=False,
                   num_devices=N_CORES)
    brep_d = nc.dram_tensor("Brep", [128, P], mybir.dt.bfloat16,
                            kind="ExternalInput").ap()
    arep_d = nc.dram_tensor("Arep", [128, HALF], mybir.dt.bfloat16,
                            kind="ExternalInput").ap()
    v_d = nc.dram_tensor("V", [128, N_JCHUNK, K], mybir.dt.bfloat16,
                         kind="ExternalInput").ap()
    out_d = nc.dram_tensor("out", [K, HALF], mybir.dt.float32,
                           kind="ExternalOutput").ap()

    with tile.TileContext(nc) as tc:
        with (
            tc.tile_pool(name="big", bufs=1) as big,
            tc.tile_pool(name="wpool", bufs=3) as wpool,
            tc.tile_pool(name="psumA", bufs=1, space="PSUM") as psumA,
            tc.tile_pool(name="psumB", bufs=1, space="PSUM") as psumB,
            tc.tile_pool(name="psacc", bufs=1, space="PSUM") as psacc,
            tc.tile_pool(name="outp", bufs=4) as outp,
        ):
            # split input DMAs so the first units' operands land early
            arep = big.tile([128, HALF], mybir.dt.bfloat16)
            nc.sync.dma_start(out=arep[:, 0:NI], in_=arep_d[:, 0:NI])
            brep = big.tile([128, P], mybir.dt.bfloat16)
            nc.sync.dma_start(out=brep[:, 0:NI], in_=brep_d[:, 0:NI])
            nc.sync.dma_start(out=brep[:, NI:1152], in_=brep_d[:, NI:1152])
            vt = big.tile([128, N_JCHUNK, K], mybir.dt.bfloat16)
            nc.sync.dma_start(out=vt, in_=v_d)
            for lo in range(1152, P, 1152):
                nc.sync.dma_start(out=brep[:, lo:lo + 1152],
                                  in_=brep_d[:, lo:lo + 1152])
            nc.sync.dma_start(out=arep[:, NI:], in_=arep_d[:, NI:])

            # per-column-band stage-2 accumulators: [band partitions, 512]
            accum = psacc.tile([128, NI], mybir.dt.float32)

            def emit_s2(jc0, size, wt):
                for idx in range(size):
                    jc = jc0 + idx
                    band = 32 * (jc % 3)   # accumulator column band
                    nc.tensor.matmul(
                        out=accum[band:band + K, :],
                        lhsT=vt[:, jc, :],
                        rhs=wt[:, idx, :],
                        start=(jc < 3), stop=(jc >= N_JCHUNK - 3),
                        tile_position=(0, band),
                        skip_group_check=True,
                    )

            def emit_fold(i0):
                # fold the 3 column bands and write out
                # (walrus: at most one PSUM operand per DVE instruction)
                t_a = outp.tile([K, NI], mybir.dt.float32, tag="fold_a")
                t_b = outp.tile([K, NI], mybir.dt.float32, tag="fold_b")
                t_c = outp.tile([K, NI], mybir.dt.float32, tag="fold_c")
                nc.vector.tensor_copy(t_a, accum[0:K, :])
                nc.vector.tensor_add(t_b, t_a, accum[32:32 + K, :])
                nc.vector.tensor_add(t_c, t_b, accum[64:64 + K, :])
                nc.sync.dma_start(out=out_d[:, i0:i0 + NI], in_=t_c)

            # stage-2 of unit g is emitted during step g+1 so the PE never
            # blocks the stage-1 matmuls that feed the next exp
            pending = None  # (jc0, size, wt, i0, last_in_itile)
            for it, (i0, wi) in enumerate(I_TILES):
                jc0 = 0
                for u, size in enumerate(UNIT_SIZES):
                    if size == 4:
                        pe_unit = psumA.tile([128, 4, NI], mybir.dt.float32,
                                             tag="unitA")
                    else:
                        pe_unit = psumB.tile([128, 3, NI], mybir.dt.float32,
                                             tag="unitB")
                    for c in range(size):
                        jc = jc0 + c
                        band = 32 * c
                        nc.tensor.matmul(
                            out=pe_unit[:, c, :],
                            lhsT=brep[band:band + 21, jc * 128:(jc + 1) * 128],
                            rhs=arep[band:band + 21, i0:i0 + wi],
                            start=True, stop=True,
                            tile_position=(band, 0),
                        )
                    wt = wpool.tile([128, 4, NI], mybir.dt.bfloat16,
                                    tag="wt")
                    nc.scalar.activation(
                        wt[:, :size, :], pe_unit[:, :size, :],
                        mybir.ActivationFunctionType.Exp,
                    )
                    if pending is not None:
                        pjc0, psize, pwt, pi0, plast = pending
                        emit_s2(pjc0, psize, pwt)
                        if plast:
                            emit_fold(pi0)
                    pending = (jc0, size, wt, i0,
                               u == len(UNIT_SIZES) - 1)
                    jc0 += size
            pjc0, psize, pwt, pi0, plast = pending
            emit_s2(pjc0, psize, pwt)
            emit_fold(pi0)

    if split_waits:
        _split_seq_waits(nc, mybir)
    return nc


def _prep_features(raw):
    """raw [K, H, W] float32 -> (Ap, Bp) each [P, 21] bf16-exact float32."""
    ys, xs = np.meshgrid(np.arange(H, dtype=np.float64),
                         np.arange(W, dtype=np.float64), indexing="ij")
    pos = np.stack([xs.ravel(), ys.ravel()], axis=-1) / SIGMA_XY       # [P, 2]
    col = raw.reshape(K, P).T.astype(np.float64) / SIGMA_RGB           # [P, 3]
    f = np.concatenate([pos, col], axis=-1)                            # [P, 5]
    f = f - f.mean(axis=0, keepdims=True)  # d2-invariant recentring
    sq = np.sum(f * f, axis=-1)
    ones = np.ones((P, 1))
    a = np.concatenate([f, -0.5 * sq[:, None], ones], axis=-1).astype(np.float32)
    b = np.concatenate([f, ones, -0.5 * sq[:, None]], axis=-1).astype(np.float32)
    ah = a.astype(bf16)
    al = (a - ah.astype(np.float32)).astype(bf16)
    bh = b.astype(bf16)
    bl = (b - bh.astype(np.float32)).astype(bf16)
    ap = np.concatenate([ah, al, ah], axis=-1)  # [P, 21] bf16
    bp = np.concatenate([bh, bh, bl], axis=-1)  # [P, 21] bf16
    return ap, bp


def _prep_core(raw, norm, half):
    """Build per-core input map. raw/norm [K, H, W] f32."""
    ap, bp = _prep_features(raw)
    i0 = half * HALF
    brep = np.zeros((128, P), bf16)
    arep = np.zeros((128, HALF), bf16)
    for c in range(4):  # replicate at all four 32-partition row bands
        brep[32 * c:32 * c + 21, :] = bp.T
        arep[32 * c:32 * c + 21, :] = ap[i0:i0 + HALF].T
    v = norm.reshape(K, P).astype(np.float32)          # [K, P]
    vtile = np.zeros((128, N_JCHUNK, K), bf16)
    # vtile[p, jc, k] = v[k, jc*128 + p]
    vtile[:, :, :] = v.T.reshape(N_JCHUNK, 128, K).transpose(1, 0, 2).astype(bf16)
    return {"Brep": brep, "Arep": arep, "V": vtile}


def _run(in_maps, trace=False):
    from concourse.bass_utils import run_bass_kernel_spmd

    if "nc" not in _CACHE:
        _CACHE["nc"] = _build()
    nc = _CACHE["nc"]
    return run_bass_kernel_spmd(nc, in_maps, core_ids=list(range(N_CORES)),
                                trace=trace)


def kernel(images_raw, images_normalized, _trace=False, _results=None):
    images_raw = np.asarray(images_raw, dtype=np.float32)
    images_normalized = np.asarray(images_normalized, dtype=np.float32)

    in_maps = []
    for c in range(N_CORES):
        n, h = divmod(c, 2)
        in_maps.append(_prep_core(images_raw[n], images_normalized[n], h))

    res = _run(in_maps, trace=_trace)
    if _results is not None:
        _results.append(res)

    out = np.zeros((N_IMG, K, P), np.float32)
    for c in range(N_CORES):
        n, h = divmod(c, 2)
        out[n][:, h * HALF:(h + 1) * HALF] = res.results[c]["out"]
    out /= out.max()
    return out.reshape(N_IMG, K, H, W)


# revision 20
# speedup vs baseline: 1.0452x; 1.0005x over previous
"""Bilateral filtering kernel for Trainium2 (8 NeuronCores, SPMD).

Problem: for each image (N=4, K=3, H=W=96, P=H*W=9216):
    f_i = (x_i/100, y_i/100, rgb_i/15) in R^5
    w[i,j] = exp(-0.5 ||f_i - f_j||^2)
    out_k[i] = sum_j w[i,j] * norm_k[j]
then out /= max(out) over the whole batch.

Sharding: core c handles image c//2, output-row half c%2 (4608 rows each).

Device algorithm per core (all P x P work on-chip, never touches HBM):
  exponent arg(i,j) = f_i.f_j - 0.5|f_i|^2 - 0.5|f_j|^2  (= -0.5 d2)
  expressed as a 7-dim dot product a_i.b_j with augmented features, and
  computed in compensated bf16 (hi/lo split -> 21-dim contraction) on the
  tensor engine: 3 row-tiled matmuls (contract 21 <= 32) per unit.
  exp() on the scalar engine reads 3-bank fp32 PSUM spans (1536 elems)
  and writes bf16 W tiles to SBUF; the scalar engine is the bottleneck
  (42.5M exps/core at 128 lanes @ 1.2 GHz ~= 332us of the ~370us total).
  Stage 2 (out = v @ w) as 3 col-tiled matmuls (M=3) accumulating into
  per-column-band regions of one PSUM bank, folded on the vector engine.
"""

import numpy as np
import ml_dtypes

bf16 = ml_dtypes.bfloat16

N_CORES = 8
N_IMG, K, H, W = 4, 3, 96, 96
P = H * W            # 9216
HALF = P // 2        # 4608
SIGMA_RGB = 15.0
SIGMA_XY = 100.0
N_JCHUNK = P // 128  # 72
# exp-unit sizes per i-tile (j-chunks per scalar-engine exp instruction).
# Asymmetric [4,3] pairs use 4+3+1(accum) = 8 PSUM banks and give bigger
# activation spans (2048/1536) than uniform 3s, amortizing the ~312-cycle
# per-instruction ACT overhead.
UNIT_SIZES = [4, 3] * 10 + [2]   # sums to 72
NI = 512             # i-tile width (fp32 PSUM: one matmul out <= 512 fp32)
I_TILES = [(i * NI, NI) for i in range(HALF // NI)]

_CACHE = {}


def _split_seq_waits(nc, mybir):
    """walrus on this build accepts only 1 sync wait on sequencer-only
    instructions (TPB_CTRL); split extras onto preceding drain carriers."""
    for fn in nc.m.functions:
        for bb in fn.blocks:
            insts = list(bb.instructions)
            out = []
            changed = False
            for ins in insts:
                si = ins.sync_info
                if si is not None and len(si.on_wait) > 1 and ins.is_sequencer_only:
                    waits = list(si.on_wait)
                    for w in waits[:-1]:
                        d = mybir.InstDrain(
                            name=nc.get_next_instruction_name(),
                            ins=[], outs=[], bass_is_fusable=False,
                        )
                        d.engine = ins.engine
                        d.sync_info = mybir.SyncInfo(on_wait=[w], on_update=[])
                        out.append(d)
                    ins.sync_info = mybir.SyncInfo(
                        on_wait=waits[-1:], on_update=list(si.on_update)
                    )
                    changed = True
                out.append(ins)
            if changed:
                bb.instructions = out


def _is_barrier_inst(ins, mybir):
    if not isinstance(ins, (mybir.InstDrain, mybir.InstEventSemaphore)):
        return False
    si = ins.sync_info
    if si is None:
        return isinstance(ins, mybir.InstDrain) and not getattr(
            ins, "is_reset_sema", False)
    refs = [w.ant_name or "" for w in si.on_wait] + [
        u.ant_name or "" for u in si.on_update]
    return bool(refs) and all(r.startswith("barrier_") for r in refs)


def _trim_barriers(nc, mybir):
    """Drop the init all-engine barrier and the post-sem-reset barrier:
    the first costs startup skew we don't need (tile sems order all real
    work), the second only delays engine halt after the PL-side semaphore
    range clear, which no other engine observes."""
    for fn in nc.m.functions:
        for bb in fn.blocks:
            insts = list(bb.instructions)
            if bb.name == "main":
                keep = [i for i in insts if not _is_barrier_inst(i, mybir)]
                if len(keep) != len(insts):
                    bb.instructions = keep
            elif bb.name.endswith("_end"):
                # find the semaphore range-clear; drop the barrier after it
                isa_idx = max((k for k, i in enumerate(insts)
                               if isinstance(i, mybir.InstISA)), default=None)
                if isa_idx is not None and isa_idx + 1 < len(insts):
                    tail = insts[isa_idx + 1:]
                    if all(_is_barrier_inst(i, mybir) for i in tail):
                        bb.instructions = insts[:isa_idx + 1]


def _build(split_waits=True):
    import concourse.bass as bass
    import concourse.tile as tile
    from concourse import mybir

    nc = bass.Bass("TRN2", target_bir_lowering=False, debug=False,
                   num_devices=N_CORES)
    brep_d = nc.dram_tensor("Brep", [128, P], mybir.dt.bfloat16,
                            kind="ExternalInput").ap()
    arep_d = nc.dram_tensor("Arep", [128, HALF], mybir.dt.bfloat16,
                            kind="ExternalInput").ap()
    v_d = nc.dram_tensor("V", [128, N_JCHUNK, K], mybir.dt.bfloat16,
                         kind="ExternalInput").ap()
    out_d = nc.dram_tensor("out", [K, HALF], mybir.dt.float32,
                           kind="ExternalOutput").ap()

    with tile.TileContext(nc) as tc:
        with (
            tc.tile_pool(name="big", bufs=1) as big,
            tc.tile_pool(name="wpool", bufs=3) as wpool,
            tc.tile_pool(name="psumA", bufs=1, space="PSUM") as psumA,
            tc.tile_pool(name="psumB", bufs=1, space="PSUM") as psumB,
            tc.tile_pool(name="psacc", bufs=1, space="PSUM") as psacc,
            tc.tile_pool(name="outp", bufs=4) as outp,
        ):
            # split input DMAs so the first units' operands land early
            arep = big.tile([128, HALF], mybir.dt.bfloat16)
            nc.sync.dma_start(out=arep[:, 0:NI], in_=arep_d[:, 0:NI])
            brep = big.tile([128, P], mybir.dt.bfloat16)
            nc.sync.dma_start(out=brep[:, 0:NI], in_=brep_d[:, 0:NI])
            nc.sync.dma_start(out=brep[:, NI:1152], in_=brep_d[:, NI:1152])
            vt = big.tile([128, N_JCHUNK, K], mybir.dt.bfloat16)
            nc.sync.dma_start(out=vt, in_=v_d)
            for lo in range(1152, P, 1152):
                nc.sync.dma_start(out=brep[:, lo:lo + 1152],
                                  in_=brep_d[:, lo:lo + 1152])
            nc.sync.dma_start(out=arep[:, NI:], in_=arep_d[:, NI:])

            # per-column-band stage-2 accumulators: [band partitions, 512]
            accum = psacc.tile([128, NI], mybir.dt.float32)

            def emit_s2(jc0, size, wt):
                for idx in range(size):
                    jc = jc0 + idx
                    band = 32 * (jc % 3)   # accumulator column band
                    nc.tensor.matmul(
                        out=accum[band:band + K, :],
                        lhsT=vt[:, jc, :],
                        rhs=wt[:, idx, :],
                        start=(jc < 3), stop=(jc >= N_JCHUNK - 3),
                        tile_position=(0, band),
                        skip_group_check=True,
                    )

            def emit_fold(i0):
                # fold the 3 column bands and write out
                # (walrus: at most one PSUM operand per DVE instruction)
                t_a = outp.tile([K, NI], mybir.dt.float32, tag="fold_a")
                t_b = outp.tile([K, NI], mybir.dt.float32, tag="fold_b")
                t_c = outp.tile([K, NI], mybir.dt.float32, tag="fold_c")
                nc.vector.tensor_copy(t_a, accum[0:K, :])
                nc.vector.tensor_add(t_b, t_a, accum[32:32 + K, :])
                nc.vector.tensor_add(t_c, t_b, accum[64:64 + K, :])
                nc.sync.dma_start(out=out_d[:, i0:i0 + NI], in_=t_c)

            # stage-2 of unit g is emitted during step g+1 so the PE never
            # blocks the stage-1 matmuls that feed the next exp
            pending = None  # (jc0, size, wt, i0, last_in_itile)
            for it, (i0, wi) in enumerate(I_TILES):
                jc0 = 0
                for u, size in enumerate(UNIT_SIZES):
                    if size == 4:
                        pe_unit = psumA.tile([128, 4, NI], mybir.dt.float32,
                                             tag="unitA")
                    else:
                        pe_unit = psumB.tile([128, 3, NI], mybir.dt.float32,
                                             tag="unitB")
                    for c in range(size):
                        jc = jc0 + c
                        band = 32 * c
                        nc.tensor.matmul(
                            out=pe_unit[:, c, :],
                            lhsT=brep[band:band + 21, jc * 128:(jc + 1) * 128],
                            rhs=arep[band:band + 21, i0:i0 + wi],
                            start=True, stop=True,
                            tile_position=(band, 0),
                        )
                    wt = wpool.tile([128, 4, NI], mybir.dt.bfloat16,
                                    tag="wt")
                    nc.scalar.activation(
                        wt[:, :size, :], pe_unit[:, :size, :],
                        mybir.ActivationFunctionType.Exp,
                    )
                    if pending is not None:
                        pjc0, psize, pwt, pi0, plast = pending
                        emit_s2(pjc0, psize, pwt)
                        if plast:
                            emit_fold(pi0)
                    pending = (jc0, size, wt, i0,
                               u == len(UNIT_SIZES) - 1)
                    jc0 += size
            pjc0, psize, pwt, pi0, plast = pending
            emit_s2(pjc0, psize, pwt)
            emit_fold(pi0)

    if split_waits:
        _trim_barriers(nc, mybir)
        _split_seq_waits(nc, mybir)
    return nc


def _prep_features(raw):
    """raw [K, H, W] float32 -> (Ap, Bp) each [P, 21] bf16-exact float32."""
    ys, xs = np.meshgrid(np.arange(H, dtype=np.float64),
                         np.arange(W, dtype=np.float64), indexing="ij")
    pos = np.stack([xs.ravel(), ys.ravel()], axis=-1) / SIGMA_XY       # [P, 2]
    col = raw.reshape(K, P).T.astype(np.float64) / SIGMA_RGB           # [P, 3]
    f = np.concatenate([pos, col], axis=-1)                            # [P, 5]
    f = f - f.mean(axis=0, keepdims=True)  # d2-invariant recentring
    sq = np.sum(f * f, axis=-1)
    ones = np.ones((P, 1))
    a = np.concatenate([f, -0.5 * sq[:, None], ones], axis=-1).astype(np.float32)
    b = np.concatenate([f, ones, -0.5 * sq[:, None]], axis=-1).astype(np.float32)
    ah = a.astype(bf16)
    al = (a - ah.astype(np.float32)).astype(bf16)
    bh = b.astype(bf16)
    bl = (b - bh.astype(np.float32)).astype(bf16)
    ap = np.concatenate([ah, al, ah], axis=-1)  # [P, 21] bf16
    bp = np.concatenate([bh, bh, bl], axis=-1)  # [P, 21] bf16
    return ap, bp


def _prep_core(raw, norm, half):
    """Build per-core input map. raw/norm [K, H, W] f32."""
    ap, bp = _prep_features(raw)
    i0 = half * HALF
    brep = np.zeros((128, P), bf16)
    arep = np.zeros((128, HALF), bf16)
    for c in range(4):  # replicate at all four 32-partition row bands
        brep[32 * c:32 * c + 21, :] = bp.T
        arep[32 * c:32 * c + 21, :] = ap[i0:i0 + HALF].T
    v = norm.reshape(K, P).astype(np.float32)          # [K, P]
    vtile = np.zeros((128, N_JCHUNK, K), bf16)
    # vtile[p, jc, k] = v[k, jc*128 + p]
    vtile[:, :, :] = v.T.reshape(N_JCHUNK, 128, K).transpose(1, 0, 2).astype(bf16)
    return {"Brep": brep, "Arep": arep, "V": vtile}


def _run(in_maps, trace=False):
    from concourse.bass_utils import run_bass_kernel_spmd

    if "nc" not in _CACHE:
        _CACHE["nc"] = _build()
    nc = _CACHE["nc"]
    return run_bass_kernel_spmd(nc, in_maps, core_ids=list(range(N_CORES)),
                                trace=trace)


def kernel(images_raw, images_normalized, _trace=False, _results=None):
    images_raw = np.asarray(images_raw, dtype=np.float32)
    images_normalized = np.asarray(images_normalized, dtype=np.float32)

    in_maps = []
    for c in range(N_CORES):
        n, h = divmod(c, 2)
        in_maps.append(_prep_core(images_raw[n], images_normalized[n], h))

    res = _run(in_maps, trace=_trace)
    if _results is not None:
        _results.append(res)

    out = np.zeros((N_IMG, K, P), np.float32)
    for c in range(N_CORES):
        n, h = divmod(c, 2)
        out[n][:, h * HALF:(h + 1) * HALF] = res.results[c]["out"]
    out /= out.max()
    return out.reshape(N_IMG, K, H, W)


# revision 22
# speedup vs baseline: 1.0538x; 1.0082x over previous
"""Bilateral filtering kernel for Trainium2 (8 NeuronCores, SPMD).

Problem: for each image (N=4, K=3, H=W=96, P=H*W=9216):
    f_i = (x_i/100, y_i/100, rgb_i/15) in R^5
    w[i,j] = exp(-0.5 ||f_i - f_j||^2)
    out_k[i] = sum_j w[i,j] * norm_k[j]
then out /= max(out) over the whole batch.

Sharding: core c handles image c//2, output-row half c%2 (4608 rows each).

Device algorithm per core (all P x P work on-chip, never touches HBM):
  exponent arg(i,j) = f_i.f_j - 0.5|f_i|^2 - 0.5|f_j|^2  (= -0.5 d2)
  expressed as a 7-dim dot product a_i.b_j with augmented features, and
  computed in compensated bf16 (hi/lo split -> 21-dim contraction) on the
  tensor engine: 3 row-tiled matmuls (contract 21 <= 32) per unit.
  exp() on the scalar engine reads 3-bank fp32 PSUM spans (1536 elems)
  and writes bf16 W tiles to SBUF; the scalar engine is the bottleneck
  (42.5M exps/core at 128 lanes @ 1.2 GHz ~= 332us of the ~370us total).
  Stage 2 (out = v @ w) as 3 col-tiled matmuls (M=3) accumulating into
  per-column-band regions of one PSUM bank, folded on the vector engine.
"""

import numpy as np
import ml_dtypes

bf16 = ml_dtypes.bfloat16

N_CORES = 8
N_IMG, K, H, W = 4, 3, 96, 96
P = H * W            # 9216
HALF = P // 2        # 4608
SIGMA_RGB = 15.0
SIGMA_XY = 100.0
N_JCHUNK = P // 128  # 72
# exp-unit sizes per i-tile (j-chunks per scalar-engine exp instruction).
# Asymmetric 4/3 units use 4+3+1(accum) = 8 PSUM banks and give bigger
# activation spans (2048/1536) than uniform 3s, amortizing the ~312-cycle
# per-instruction ACT overhead. Units strictly alternate between the two
# PSUM slots (A=4 banks, B=3 banks) regardless of size — consecutive
# same-slot units would serialize stage-1 behind the previous exp.
# Nine 4s + twelve 3s = 72; the 4s sit on even (A-slot) indices.
UNIT_SIZES = [4, 3] * 9 + [3, 3, 3]   # sums to 72, len 21
NI = 512             # i-tile width (fp32 PSUM: one matmul out <= 512 fp32)
I_TILES = [(i * NI, NI) for i in range(HALF // NI)]

_CACHE = {}


def _split_seq_waits(nc, mybir):
    """walrus on this build accepts only 1 sync wait on sequencer-only
    instructions (TPB_CTRL); split extras onto preceding drain carriers."""
    for fn in nc.m.functions:
        for bb in fn.blocks:
            insts = list(bb.instructions)
            out = []
            changed = False
            for ins in insts:
                si = ins.sync_info
                if si is not None and len(si.on_wait) > 1 and ins.is_sequencer_only:
                    waits = list(si.on_wait)
                    for w in waits[:-1]:
                        d = mybir.InstDrain(
                            name=nc.get_next_instruction_name(),
                            ins=[], outs=[], bass_is_fusable=False,
                        )
                        d.engine = ins.engine
                        d.sync_info = mybir.SyncInfo(on_wait=[w], on_update=[])
                        out.append(d)
                    ins.sync_info = mybir.SyncInfo(
                        on_wait=waits[-1:], on_update=list(si.on_update)
                    )
                    changed = True
                out.append(ins)
            if changed:
                bb.instructions = out


def _is_barrier_inst(ins, mybir):
    if not isinstance(ins, (mybir.InstDrain, mybir.InstEventSemaphore)):
        return False
    si = ins.sync_info
    if si is None:
        return isinstance(ins, mybir.InstDrain) and not getattr(
            ins, "is_reset_sema", False)
    refs = [w.ant_name or "" for w in si.on_wait] + [
        u.ant_name or "" for u in si.on_update]
    return bool(refs) and all(r.startswith("barrier_") for r in refs)


def _trim_barriers(nc, mybir):
    """Drop the init all-engine barrier and the post-sem-reset barrier:
    the first costs startup skew we don't need (tile sems order all real
    work), the second only delays engine halt after the PL-side semaphore
    range clear, which no other engine observes."""
    for fn in nc.m.functions:
        for bb in fn.blocks:
            insts = list(bb.instructions)
            if bb.name == "main":
                keep = [i for i in insts if not _is_barrier_inst(i, mybir)]
                if len(keep) != len(insts):
                    bb.instructions = keep
            elif bb.name.endswith("_end"):
                # find the semaphore range-clear; drop the barrier after it
                isa_idx = max((k for k, i in enumerate(insts)
                               if isinstance(i, mybir.InstISA)), default=None)
                if isa_idx is not None and isa_idx + 1 < len(insts):
                    tail = insts[isa_idx + 1:]
                    if all(_is_barrier_inst(i, mybir) for i in tail):
                        bb.instructions = insts[:isa_idx + 1]


def _build(split_waits=True):
    import concourse.bass as bass
    import concourse.tile as tile
    from concourse import mybir

    nc = bass.Bass("TRN2", target_bir_lowering=False, debug=False,
                   num_devices=N_CORES)
    brep_d = nc.dram_tensor("Brep", [128, P], mybir.dt.bfloat16,
                            kind="ExternalInput").ap()
    arep_d = nc.dram_tensor("Arep", [128, HALF], mybir.dt.bfloat16,
                            kind="ExternalInput").ap()
    v_d = nc.dram_tensor("V", [128, N_JCHUNK, K], mybir.dt.bfloat16,
                         kind="ExternalInput").ap()
    out_d = nc.dram_tensor("out", [K, HALF], mybir.dt.float32,
                           kind="ExternalOutput").ap()

    with tile.TileContext(nc) as tc:
        with (
            tc.tile_pool(name="big", bufs=1) as big,
            tc.tile_pool(name="wpool", bufs=3) as wpool,
            tc.tile_pool(name="psumA", bufs=1, space="PSUM") as psumA,
            tc.tile_pool(name="psumB", bufs=1, space="PSUM") as psumB,
            tc.tile_pool(name="psacc", bufs=1, space="PSUM") as psacc,
            tc.tile_pool(name="outp", bufs=4) as outp,
        ):
            # split input DMAs so the first units' operands land early
            arep = big.tile([128, HALF], mybir.dt.bfloat16)
            nc.sync.dma_start(out=arep[:, 0:NI], in_=arep_d[:, 0:NI])
            brep = big.tile([128, P], mybir.dt.bfloat16)
            nc.sync.dma_start(out=brep[:, 0:NI], in_=brep_d[:, 0:NI])
            nc.sync.dma_start(out=brep[:, NI:1152], in_=brep_d[:, NI:1152])
            vt = big.tile([128, N_JCHUNK, K], mybir.dt.bfloat16)
            nc.sync.dma_start(out=vt, in_=v_d)
            for lo in range(1152, P, 1152):
                nc.sync.dma_start(out=brep[:, lo:lo + 1152],
                                  in_=brep_d[:, lo:lo + 1152])
            nc.sync.dma_start(out=arep[:, NI:], in_=arep_d[:, NI:])

            # per-column-band stage-2 accumulators: [band partitions, 512]
            accum = psacc.tile([128, NI], mybir.dt.float32)

            def emit_s2(jc0, size, wt):
                for idx in range(size):
                    jc = jc0 + idx
                    band = 32 * (jc % 3)   # accumulator column band
                    nc.tensor.matmul(
                        out=accum[band:band + K, :],
                        lhsT=vt[:, jc, :],
                        rhs=wt[:, idx, :],
                        start=(jc < 3), stop=(jc >= N_JCHUNK - 3),
                        tile_position=(0, band),
                        skip_group_check=True,
                    )

            def emit_fold(i0):
                # fold the 3 column bands and write out
                # (walrus: at most one PSUM operand per DVE instruction)
                t_a = outp.tile([K, NI], mybir.dt.float32, tag="fold_a")
                t_b = outp.tile([K, NI], mybir.dt.float32, tag="fold_b")
                t_c = outp.tile([K, NI], mybir.dt.float32, tag="fold_c")
                nc.vector.tensor_copy(t_a, accum[0:K, :])
                nc.vector.tensor_add(t_b, t_a, accum[32:32 + K, :])
                nc.vector.tensor_add(t_c, t_b, accum[64:64 + K, :])
                nc.sync.dma_start(out=out_d[:, i0:i0 + NI], in_=t_c)

            # stage-2 of unit g is emitted during step g+1 so the PE never
            # blocks the stage-1 matmuls that feed the next exp
            pending = None  # (jc0, size, wt, i0, last_in_itile)
            for it, (i0, wi) in enumerate(I_TILES):
                jc0 = 0
                for u, size in enumerate(UNIT_SIZES):
                    if u % 2 == 0:
                        pe_unit = psumA.tile([128, 4, NI], mybir.dt.float32,
                                             tag="unitA")
                    else:
                        pe_unit = psumB.tile([128, 3, NI], mybir.dt.float32,
                                             tag="unitB")
                    for c in range(size):
                        jc = jc0 + c
                        band = 32 * c
                        nc.tensor.matmul(
                            out=pe_unit[:, c, :],
                            lhsT=brep[band:band + 21, jc * 128:(jc + 1) * 128],
                            rhs=arep[band:band + 21, i0:i0 + wi],
                            start=True, stop=True,
                            tile_position=(band, 0),
                        )
                    wt = wpool.tile([128, 4, NI], mybir.dt.bfloat16,
                                    tag="wt")
                    nc.scalar.activation(
                        wt[:, :size, :], pe_unit[:, :size, :],
                        mybir.ActivationFunctionType.Exp,
                    )
                    if pending is not None:
                        pjc0, psize, pwt, pi0, plast = pending
                        emit_s2(pjc0, psize, pwt)
                        if plast:
                            emit_fold(pi0)
                    pending = (jc0, size, wt, i0,
                               u == len(UNIT_SIZES) - 1)
                    jc0 += size
            pjc0, psize, pwt, pi0, plast = pending
            emit_s2(pjc0, psize, pwt)
            emit_fold(pi0)

    if split_waits:
        _trim_barriers(nc, mybir)
        _split_seq_waits(nc, mybir)
    return nc


def _prep_features(raw):
    """raw [K, H, W] float32 -> (Ap, Bp) each [P, 21] bf16-exact float32."""
    ys, xs = np.meshgrid(np.arange(H, dtype=np.float64),
                         np.arange(W, dtype=np.float64), indexing="ij")
    pos = np.stack([xs.ravel(), ys.ravel()], axis=-1) / SIGMA_XY       # [P, 2]
    col = raw.reshape(K, P).T.astype(np.float64) / SIGMA_RGB           # [P, 3]
    f = np.concatenate([pos, col], axis=-1)                            # [P, 5]
    f = f - f.mean(axis=0, keepdims=True)  # d2-invariant recentring
    sq = np.sum(f * f, axis=-1)
    ones = np.ones((P, 1))
    a = np.concatenate([f, -0.5 * sq[:, None], ones], axis=-1).astype(np.float32)
    b = np.concatenate([f, ones, -0.5 * sq[:, None]], axis=-1).astype(np.float32)
    ah = a.astype(bf16)
    al = (a - ah.astype(np.float32)).astype(bf16)
    bh = b.astype(bf16)
    bl = (b - bh.astype(np.float32)).astype(bf16)
    ap = np.concatenate([ah, al, ah], axis=-1)  # [P, 21] bf16
    bp = np.concatenate([bh, bh, bl], axis=-1)  # [P, 21] bf16
    return ap, bp


def _prep_core(raw, norm, half):
    """Build per-core input map. raw/norm [K, H, W] f32."""
    ap, bp = _prep_features(raw)
    i0 = half * HALF
    brep = np.zeros((128, P), bf16)
    arep = np.zeros((128, HALF), bf16)
    for c in range(4):  # replicate at all four 32-partition row bands
        brep[32 * c:32 * c + 21, :] = bp.T
        arep[32 * c:32 * c + 21, :] = ap[i0:i0 + HALF].T
    v = norm.reshape(K, P).astype(np.float32)          # [K, P]
    vtile = np.zeros((128, N_JCHUNK, K), bf16)
    # vtile[p, jc, k] = v[k, jc*128 + p]
    vtile[:, :, :] = v.T.reshape(N_JCHUNK, 128, K).transpose(1, 0, 2).astype(bf16)
    return {"Brep": brep, "Arep": arep, "V": vtile}


def _run(in_maps, trace=False):
    from concourse.bass_utils import run_bass_kernel_spmd

    if "nc" not in _CACHE:
        _CACHE["nc"] = _build()
    nc = _CACHE["nc"]
    return run_bass_kernel_spmd(nc, in_maps, core_ids=list(range(N_CORES)),
                                trace=trace)


def kernel(images_raw, images_normalized, _trace=False, _results=None):
    images_raw = np.asarray(images_raw, dtype=np.float32)
    images_normalized = np.asarray(images_normalized, dtype=np.float32)

    in_maps = []
    for c in range(N_CORES):
        n, h = divmod(c, 2)
        in_maps.append(_prep_core(images_raw[n], images_normalized[n], h))

    res = _run(in_maps, trace=_trace)
    if _results is not None:
        _results.append(res)

    out = np.zeros((N_IMG, K, P), np.float32)
    for c in range(N_CORES):
        n, h = divmod(c, 2)
        out[n][:, h * HALF:(h + 1) * HALF] = res.results[c]["out"]
    out /= out.max()
    return out.reshape(N_IMG, K, H, W)
